# revision 1
# baseline (speedup 1.0000x reference)
"""Llama GQA attention layer (B=1, S=2048, D=4096, H=32, KVH=8, DH=128) on 8 trn2 cores.

Sharding: tensor-parallel over heads. Core c owns Q heads [4c, 4c+4) and KV head c:
  Wq[:, c*512:(c+1)*512], Wk/Wv[:, c*128:(c+1)*128], Wo rows [c*512:(c+1)*512].
Each core computes a partial [2048, 4096] output (row-parallel Wo); host sums the
8 partials (the all-reduce of the row-parallel sharding).

Kernel layout strategy (per core):
  - X^T [4096, 2048] streamed; projections computed as Q^T/K^T/V^T [dh, s] via
    PSUM accumulation over 32 d-tiles (fp32r matmuls, full PE rate at N=512).
  - RoPE applied on PSUM evacuation (DVE, partition-half shuffle).
  - V^T transposed to V natural [s, dh] via PE-transpose (needed as PV stationary).
  - Attention with scores TRANSPOSED: S^T[k, q] tiles [128, 512] so softmax sums
    over keys become ones-vector matmuls; exp on ACT (no max subtraction - scores
    are O(10), exp is safe); causal sparsity by skipping fully-masked key tiles;
    diagonal tiles masked multiplicatively with 4 static 0/1 tiles.
  - Softmax normalization: recip of sums row [1,512] broadcast across partitions
    via a K=1 ones matmul, then one DVE mul per attn^T tile.
  - Output projection accumulating over the 4 head-blocks, streamed out.
"""

import numpy as np

import concourse.bass as bass
import concourse.bacc as bacc
import concourse.mybir as mybir
import concourse.tile as tile
from concourse.bass_utils import run_bass_kernel_spmd

S = 2048
D = 4096
H = 32
KVH = 8
DH = 128
NCORES = 8
HPC = H // NCORES            # 4 query heads per core
QC = HPC * DH                # 512 projection cols per core
SCALE = float(DH) ** -0.5
NT_D = D // 128              # 32 contraction tiles
NCH = S // 512               # 4 sequence chunks
FP32 = mybir.dt.float32
FP32R = mybir.dt.float32r
BF16 = mybir.dt.bfloat16
AF = mybir.ActivationFunctionType

import os as _os
MMDT = {"bf16": BF16, "fp32r": FP32R}[_os.environ.get("KERNEL_MM_DTYPE", "bf16")]


def _np_mmdt():
    import ml_dtypes
    return {BF16: ml_dtypes.bfloat16, FP32R: np.float32}[MMDT]


def _r(ap):
    return ap


def _emit(nc, tc, io, mode, phases="ABC"):
    """mode: 'causal' (sparse, static diag masks), 'dense' (all tiles, no mask),
    'masked' (all tiles, additive mask streamed from DRAM)."""
    from contextlib import ExitStack

    xt_d, wq_d, wk_d, wv_d, wo_d, cos_d, sin_d, msk_d, id_d, on_d, out_d = io

    with ExitStack() as top:
        ep = top.enter_context  # persistent pools

        # ---------- persistent SBUF (whole kernel) ----------
        pers = ep(tc.tile_pool(name="pers", bufs=1))
        qt = pers.tile([128, HPC * S], MMDT, name="qt")        # Q^T, head h at [:, h*S:(h+1)*S]
        kt = pers.tile([128, S], MMDT, name="kt")              # K^T
        vn = pers.tile([128, S], MMDT, name="vn")              # V natural, tile t at [:, 128t:128t+128]
        at = pers.tile([128, HPC * S], MMDT, name="at")        # attn^T
        ones_c = pers.tile([128, 1], MMDT, name="ones_c")
        ones_r = pers.tile([1, 128], FP32, name="ones_r")
        msk_sb = pers.tile([128, 4 * 512], MMDT, name="msk_sb")

        # ================= Phase A: projections =================
        with ExitStack() as pa:
            e = pa.enter_context
            wpool = e(tc.tile_pool(name="wpool", bufs=1))
            id_sb = wpool.tile([128, 128], MMDT, name="id_sb")
            nc.sync.dma_start(id_sb[:], id_d[:])
            cs_sb = wpool.tile([128, S], FP32, name="cs_sb")
            sn_sb = wpool.tile([128, S], FP32, name="sn_sb")
            xpool = e(tc.tile_pool(name="xpool", bufs=3))
            tpool = e(tc.tile_pool(name="tpool", bufs=2))
            psum = e(tc.tile_pool(name="psumA", bufs=1, space=bass.MemorySpace.PSUM))

            # startup order: the tiles gating the first matmuls go first,
            # then the first xt pair, then everything else
            wq_t2 = [wpool.tile([128, 2 * QC], MMDT, name=f"wq2_{i}")
                     for i in range(NT_D // 2)]
            wk_t8 = [wpool.tile([128, 8 * DH], MMDT, name=f"wk8_{i}")
                     for i in range(NT_D // 8)]
            wv_t8 = [wpool.tile([128, 8 * DH], MMDT, name=f"wv8_{i}")
                     for i in range(NT_D // 8)]
            nc.sync.dma_start(wq_t2[0][:], wq_d[:, 0:2 * QC])
            nc.sync.dma_start(wk_t8[0][:], wk_d[:, 0:8 * DH])
            nc.sync.dma_start(wv_t8[0][:], wv_d[:, 0:8 * DH])
            xt_first = [xpool.tile([128, 1024], MMDT, tag="xt", bufs=4,
                                   name=f"xtf{i}") for i in range(2)]
            for i, x in enumerate(xt_first):
                nc.sync.dma_start(x[:], xt_d[:, i * 1024:(i + 1) * 1024])
            nc.sync.dma_start(ones_c[:], on_d[:])
            nc.vector.memset(ones_r[:], 1.0)
            if mode == "causal":
                nc.sync.dma_start(msk_sb[:], msk_d[:])
            for i in range(1, NT_D // 2):
                nc.sync.dma_start(wq_t2[i][:], wq_d[:, i * 2 * QC:(i + 1) * 2 * QC])
            for i in range(1, NT_D // 8):
                nc.sync.dma_start(wk_t8[i][:], wk_d[:, i * 8 * DH:(i + 1) * 8 * DH])
                nc.sync.dma_start(wv_t8[i][:], wv_d[:, i * 8 * DH:(i + 1) * 8 * DH])
            nc.sync.dma_start(cs_sb[:], cos_d[:])
            nc.sync.dma_start(sn_sb[:], sin_d[:])

            def wq_ap(dt_, h):
                return wq_t2[dt_ // 2][:, (dt_ % 2) * QC + h * 128:
                                       (dt_ % 2) * QC + (h + 1) * 128]

            def wk_ap(dt_):
                return wk_t8[dt_ // 8][:, (dt_ % 8) * DH:(dt_ % 8 + 1) * DH]

            def wv_ap(dt_):
                return wv_t8[dt_ // 8][:, (dt_ % 8) * DH:(dt_ % 8 + 1) * DH]

            def rope_evac(src_ps, dest, ci):
                cs = cs_sb[:, ci * 512:(ci + 1) * 512]
                sn = sn_sb[:, ci * 512:(ci + 1) * 512]
                t1 = tpool.tile([128, 512], FP32, tag="t1", bufs=2)
                t2 = tpool.tile([128, 512], FP32, tag="t2", bufs=2)
                nc.vector.tensor_mul(t1[:], src_ps[:], cs)
                nc.vector.tensor_mul(t2[0:64, :], src_ps[64:128, :], sn[0:64, :])
                nc.vector.tensor_mul(t2[64:128, :], src_ps[0:64, :], sn[64:128, :])
                nc.vector.tensor_sub(dest[0:64, :], t1[0:64, :], t2[0:64, :])
                nc.vector.tensor_add(dest[64:128, :], t1[64:128, :], t2[64:128, :])

            for ci in range(NCH):
                acc = [psum.tile([128, 512], FP32, tag="acc", bufs=6,
                                 name=f"acc{ci}_{b}") for b in range(6)]
                for i in range(NT_D // 2):
                    if ci == 0 and i < 2:
                        xt_t = xt_first[i]
                    else:
                        xt_t = xpool.tile([128, 1024], MMDT, tag="xt", bufs=4)
                        o = (ci * NT_D + 2 * i) * 512
                        nc.sync.dma_start(xt_t[:], xt_d[:, o:o + 1024])
                    for half in range(2):
                        dt_ = 2 * i + half
                        st = dt_ == 0
                        sp = dt_ == NT_D - 1
                        rhs = xt_t[:, half * 512:(half + 1) * 512]
                        for h in range(HPC):
                            nc.tensor.matmul(acc[h][:], wq_ap(dt_, h), rhs,
                                             start=st, stop=sp)
                        nc.tensor.matmul(acc[4][:], wk_ap(dt_), rhs,
                                         start=st, stop=sp)
                        nc.tensor.matmul(acc[5][:], wv_ap(dt_), rhs,
                                         start=st, stop=sp)
                for h in range(HPC):
                    rope_evac(acc[h], qt[:, h * S + ci * 512:h * S + (ci + 1) * 512], ci)
                rope_evac(acc[4], kt[:, ci * 512:(ci + 1) * 512], ci)
                # V: plain evac then PE-transpose each 128 block to natural layout
                vt_t = tpool.tile([128, 512], MMDT, tag="vt", bufs=2)
                nc.scalar.copy(vt_t[:], acc[5][:])
                for i in range(4):
                    ps_tr = psum.tile([128, 128], MMDT, tag="tr", bufs=2,
                                      name=f"tr{ci}_{i}")
                    nc.tensor.transpose(ps_tr[:], vt_t[:, i * 128:(i + 1) * 128], id_sb[:])
                    s0 = (ci * 4 + i) * 128
                    nc.vector.tensor_copy(vn[:, s0:s0 + 128], ps_tr[:])

        if "B" not in phases:
            return

        # ================= Phase B: attention =================
        with ExitStack() as pb:
            e = pb.enter_context
            ppool = e(tc.tile_pool(name="ppool", bufs=4))
            npool = e(tc.tile_pool(name="npool", bufs=2))
            mpool = e(tc.tile_pool(name="mpool", bufs=4))
            psum = e(tc.tile_pool(name="psumB", bufs=1, space=bass.MemorySpace.PSUM))

            for ci in range(NCH):
                n_sk = 4 * (ci + 1) if mode == "causal" else S // 128
                for h in range(HPC):
                    ps_pv = psum.tile([128, 512], FP32, tag="pv", bufs=2,
                                      name=f"pv{ci}_{h}")
                    ps_sm = psum.tile([1, 512], FP32, tag="sm", bufs=2,
                                      name=f"sm{ci}_{h}")
                    qs = qt[:, h * S + ci * 512:h * S + (ci + 1) * 512]
                    for sk in range(n_sk):
                        ps_sc = psum.tile([128, 512], FP32, tag="sc", bufs=2,
                                          name=f"sc{ci}_{h}_{sk}")
                        nc.tensor.matmul(ps_sc[:], _r(kt[:, sk * 128:(sk + 1) * 128]),
                                         _r(qs), start=True, stop=True)
                        p = ppool.tile([128, 512], MMDT, tag="p", bufs=4)
                        if mode == "masked":
                            mt = mpool.tile([128, 512], FP32, tag="mt", bufs=4)
                            nc.sync.dma_start(
                                mt[:], msk_d[sk * 128:(sk + 1) * 128,
                                             ci * 512:(ci + 1) * 512])
                            nc.vector.tensor_scalar_mul(p[:], ps_sc[:], SCALE)
                            nc.vector.tensor_add(p[:], p[:], mt[:])
                            nc.scalar.activation(p[:], p[:], AF.Exp)
                        else:
                            nc.scalar.activation(p[:], ps_sc[:], AF.Exp, scale=SCALE)
                            if mode == "causal" and sk >= 4 * ci:
                                j = sk - 4 * ci
                                nc.vector.tensor_mul(
                                    p[:], p[:], msk_sb[:, j * 512:(j + 1) * 512])
                        st = sk == 0
                        sp = sk == n_sk - 1
                        nc.tensor.matmul(ps_pv[:], _r(vn[:, sk * 128:(sk + 1) * 128]),
                                         _r(p[:]), start=st, stop=sp)
                        nc.tensor.matmul(ps_sm[:], _r(ones_c[:]), _r(p[:]),
                                         start=st, stop=sp)
                    # normalize: 1/sums broadcast over partitions via K=1 matmul
                    rc = npool.tile([1, 512], FP32, tag="rc", bufs=2)
                    rs = npool.tile([1, 512], FP32, tag="rs", bufs=2)
                    nc.vector.reciprocal_approx_accurate(rc[:], ps_sm[:], rs[:])
                    ps_bc = psum.tile([128, 512], FP32, tag="bc", bufs=2,
                                      name=f"bc{ci}_{h}")
                    nc.tensor.matmul(ps_bc[:], ones_r[:], rc[:], start=True, stop=True)
                    rb = npool.tile([128, 512], FP32, tag="rb", bufs=2)
                    nc.scalar.copy(rb[:], ps_bc[:])
                    nc.vector.tensor_mul(at[:, h * S + ci * 512:h * S + (ci + 1) * 512],
                                         ps_pv[:], rb[:])

        if "C" not in phases:
            return
        # ================= Phase C: output projection =================
        with ExitStack() as pc:
            e = pc.enter_context
            wopool = e(tc.tile_pool(name="wopool", bufs=8))
            opool = e(tc.tile_pool(name="opool", bufs=4))
            psum = e(tc.tile_pool(name="psumC", bufs=1, space=bass.MemorySpace.PSUM))
            for op_ in range(D // 1024):
                wt = []
                for odh in range(2):
                    od = 2 * op_ + odh
                    w = wopool.tile([128, HPC * 512], MMDT, tag="wo", bufs=4)
                    nc.sync.dma_start(w[:], wo_d[:, od * HPC * 512:
                                                 (od + 1) * HPC * 512])
                    wt.append(w)
                for sb in range(S // 128):
                    ob = opool.tile([128, 1024], FP32, tag="ob", bufs=4)
                    for odh in range(2):
                        ps_o = psum.tile([128, 512], FP32, tag="oo", bufs=4,
                                         name=f"oo{op_}_{sb}_{odh}")
                        for h in range(HPC):
                            nc.tensor.matmul(
                                ps_o[:],
                                at[:, h * S + sb * 128:h * S + (sb + 1) * 128],
                                wt[odh][:, h * 512:(h + 1) * 512],
                                start=(h == 0), stop=(h == HPC - 1))
                        nc.vector.tensor_copy(ob[:, odh * 512:(odh + 1) * 512],
                                              ps_o[:])
                    nc.sync.dma_start(out_d[sb * 128:(sb + 1) * 128,
                                            op_ * 1024:(op_ + 1) * 1024], ob[:])


def build(mode="causal", phases="ABC"):
    nc = bacc.Bacc("TRN2", target_bir_lowering=False, debug=False,
                   num_devices=NCORES)
    xt_d = nc.dram_tensor("xt", [128, NCH * NT_D * 512], MMDT, kind="ExternalInput").ap()
    wq_d = nc.dram_tensor("wq", [128, NT_D * QC], MMDT, kind="ExternalInput").ap()
    wk_d = nc.dram_tensor("wk", [128, NT_D * DH], MMDT, kind="ExternalInput").ap()
    wv_d = nc.dram_tensor("wv", [128, NT_D * DH], MMDT, kind="ExternalInput").ap()
    wo_d = nc.dram_tensor("wo", [128, (D // 512) * HPC * 512], MMDT, kind="ExternalInput").ap()
    cos_d = nc.dram_tensor("cost", [DH, S], FP32, kind="ExternalInput").ap()
    sin_d = nc.dram_tensor("sint", [DH, S], FP32, kind="ExternalInput").ap()
    # causal: [512, 512] = 4 stacked 0/1 tiles; masked: [S, S] additive mask^T
    mshape2 = [S, S] if mode == "masked" else [128, 4 * 512]
    msk_d = nc.dram_tensor("msk", mshape2, FP32 if mode == "masked" else MMDT, kind="ExternalInput").ap()
    id_d = nc.dram_tensor("ident", [128, 128], MMDT, kind="ExternalInput").ap()
    on_d = nc.dram_tensor("ones", [128, 1], MMDT, kind="ExternalInput").ap()
    out_d = nc.dram_tensor("out", [S, D], FP32, kind="ExternalOutput").ap()
    io = (xt_d, wq_d, wk_d, wv_d, wo_d, cos_d, sin_d, msk_d, id_d, on_d, out_d)
    with tile.TileContext(nc) as tc:
        _emit(nc, tc, io, mode, phases)
    nc.compile()
    return nc


_CACHE = {}
RUN_KWARGS = {}   # extra kwargs for run_bass_kernel_spmd (e.g. trace=True)
LAST = None       # last BassKernelResults (for exec_time_ns inspection)


def _causal_ref_mask():
    neg = np.finfo(np.float32).min
    m = np.where(np.tril(np.ones((S, S), dtype=bool)), 0.0, neg)
    return m.astype(np.float32)


def _tile_rows(w):
    # [T*128, C] -> [128, T*C] with d-tile blocks along free dim
    t = w.shape[0] // 128
    return np.ascontiguousarray(
        w.reshape(t, 128, w.shape[1]).transpose(1, 0, 2).reshape(128, -1))


def _tile_wo(w):
    # [512, D] -> [128, (od, h) blocks]: block (h, od) at [p, od*2048 + h*512]
    return np.ascontiguousarray(
        w.reshape(HPC, 128, D // 512, 512).transpose(1, 2, 0, 3).reshape(128, -1))


def make_in_maps(hidden_states, cos, sin, attention_mask, Wq, Wk, Wv, Wo, mode):
    mdt = _np_mmdt()
    xtf = np.ascontiguousarray(hidden_states.reshape(S, D).T).astype(mdt)
    # tiled (ci, dt) layout: [128, (ci*32+dt)*512 + s]
    xt = np.ascontiguousarray(
        xtf.reshape(NT_D, 128, NCH, 512).transpose(1, 2, 0, 3).reshape(128, -1))
    cost = np.ascontiguousarray(cos.T).astype(np.float32)
    sint = np.ascontiguousarray(sin.T).astype(np.float32)
    ident = np.eye(128, dtype=mdt)
    if mode == "masked":
        msk = np.ascontiguousarray(attention_mask.reshape(S, S).T).astype(np.float32)
    else:
        # 4 diagonal 0/1 tiles: tile j valid where 128*j + k <= q  (k:[128], q:[512])
        j = np.arange(4)[:, None, None]
        k = np.arange(128)[None, :, None]
        q = np.arange(512)[None, None, :]
        msk = np.ascontiguousarray((128 * j + k <= q).astype(mdt)
                                   .transpose(1, 0, 2).reshape(128, 2048))
    in_maps = []
    for c in range(NCORES):
        in_maps.append({
            "xt": xt,
            "wq": _tile_rows(np.asarray(Wq[:, c * QC:(c + 1) * QC]).astype(mdt)),
            "wk": _tile_rows(np.asarray(Wk[:, c * DH:(c + 1) * DH]).astype(mdt)),
            "wv": _tile_rows(np.asarray(Wv[:, c * DH:(c + 1) * DH]).astype(mdt)),
            "wo": _tile_wo(np.asarray(Wo[c * QC:(c + 1) * QC, :]).astype(mdt)),
            "cost": cost, "sint": sint, "msk": msk, "ident": ident,
            "ones": np.ones((128, 1), dtype=mdt),
        })
    return in_maps


def pick_mode(attention_mask):
    am = np.asarray(attention_mask).reshape(S, S)
    if np.array_equal(am, _causal_ref_mask()):
        return "causal"
    if not np.any(am):
        return "dense"
    return "masked"


def kernel(hidden_states, cos, sin, attention_mask, Wq, Wk, Wv, Wo, **kwargs):
    mode = pick_mode(attention_mask)
    ck = (mode, str(MMDT))
    if ck not in _CACHE:
        _CACHE[ck] = build(mode)
    nc = _CACHE[ck]
    in_maps = make_in_maps(hidden_states, cos, sin, attention_mask,
                           Wq, Wk, Wv, Wo, mode)
    res = run_bass_kernel_spmd(nc, in_maps, core_ids=list(range(NCORES)),
                               **RUN_KWARGS)
    global LAST
    LAST = res
    total = np.zeros((S, D), dtype=np.float64)
    for c in range(NCORES):
        total += res.results[c]["out"].astype(np.float64)
    return total.astype(np.float32).reshape(1, S, D)

